# revision 2
# baseline (speedup 1.0000x reference)
"""Trainium2 Bass kernel v4: MultiHeadAttention with relative position embeddings.

v3 (transposed-S + SBUF skew + ones-column softmax) reworked to cut PE
sequencer pressure: i-tiles processed in pairs so each k.q matmul covers 256
query columns, PSUM banks hold [128(j), 2(jt), 256(i)], and the emission order
packs S_T of both heads back-to-back so exp/PV overlap the other head's
matmuls.
"""
import sys
for p in ("/opt/trn_rl_repo",):
    if p not in sys.path:
        sys.path.append(p)

import numpy as np

import concourse.bass as bass
from concourse import mybir, bacc
from concourse.tile import TileContext
from concourse.masks import make_identity
from concourse.bass_utils import run_bass_kernel_spmd

F16 = mybir.dt.float16
BF16 = mybir.dt.bfloat16
F32 = mybir.dt.float32
FP8 = mybir.dt.float8e4
DR = mybir.MatmulPerfMode.DoubleRow
NPF16 = np.float16

B, L, D, H, HD = 2, 2048, 1024, 16, 64
MAX_LEN = 2048
NCORES = 8
EPC = 128            # head-dims per core (2 heads x 64)
BL = B * L           # 4096 flattened (b, l)
NT = L // 128        # 16 l-tiles per sequence
BAND = 2176          # QE band width per l-tile (2048 + 127, padded +1)
RELW = 4096          # rel table padded from 4095

Exp = mybir.ActivationFunctionType.Exp


def build_bass(replicas=1):
    nc = bacc.Bacc(None)
    xT = nc.declare_dram_parameter("xT", [D, BL], F16, False)
    wq = nc.declare_dram_parameter("wqT8", [D, EPC], F16, False)
    wk = nc.declare_dram_parameter("wkT", [D, EPC], F16, False)
    wv = nc.declare_dram_parameter("wvT", [D, EPC], F16, False)
    wo = nc.declare_dram_parameter("woT", [EPC, D], F16, False)
    rel = nc.declare_dram_parameter("rel8T", [HD, RELW], F16, False)
    bq = nc.declare_dram_parameter("bq8", [EPC, 1], F32, False)
    out = nc.declare_dram_parameter("out", [BL, D], F16, True)

    with TileContext(nc) as tc:
      for _rep in range(replicas):
        with (
            tc.tile_pool(name="singles", bufs=1) as singles,
            tc.tile_pool(name="qe", bufs=5) as qep,
            tc.tile_pool(name="rsh", bufs=6) as rshp,
            tc.tile_pool(name="pbt", bufs=3) as pbtp,
            tc.tile_pool(name="attn", bufs=4) as attnp,
            tc.tile_pool(name="attT", bufs=2) as attTp,
            tc.tile_pool(name="osb", bufs=3) as outp,
            tc.tile_pool(name="small", bufs=8) as small,
        ):
            # ---- persistent tiles + loads ----
            qT = singles.tile([128, BL], F16, tag="qT")
            kT = singles.tile([128, BL], F16, tag="kT")
            # fp8 q/k packed [32(hd%32), 2(hd half), ...] per head at partition
            # offset 32*h, for DoubleRow k.q matmuls
            q8p = singles.tile([64, 2, BL], FP8, tag="q8p")
            k8p = singles.tile([64, 2, BL], FP8, tag="k8p")
            vsb = singles.tile([128, BL // 128, 2, 65], F16, tag="v")
            relsb = singles.tile([128, RELW], F16, tag="rel")
            wosb = singles.tile([128, D], F16, tag="wo")
            ident = singles.tile([128, 128], F16, tag="ident")
            bq_s = singles.tile([128, 1], F32, tag="bq")
            wq_s = singles.tile([128, 8, 128], F16, tag="wq")
            wk_s = singles.tile([128, 8, 128], F16, tag="wk")
            wv_s = singles.tile([128, 8, 128], F16, tag="wv")

            make_identity(nc, ident)
            nc.gpsimd.memset(vsb, 1.0)   # ones column at [..., 64] survives

            # ---- phase A: q/k/v projections ----
            with (
                tc.tile_pool(name="xin", bufs=8) as xpool,
                tc.tile_pool(name="psA", bufs=2, space="PSUM") as psA,
                tc.tile_pool(name="psV", bufs=2, space="PSUM") as psV,
            ):
                xts = []
                for kk in range(8):
                    xt = xpool.tile([128, BL], F16, tag="xt")
                    eng = nc.sync if kk < 4 else nc.gpsimd
                    eng.dma_start(out=xt, in_=xT[kk * 128:(kk + 1) * 128, :])
                    xts.append(xt)
                # weights/rel after x so the first projection isn't starved
                for w_s, w_d in ((wq_s, wq), (wk_s, wk), (wv_s, wv)):
                    nc.gpsimd.dma_start(out=w_s, in_=w_d[:, :].rearrange("(k p) e -> p k e", p=128))
                nc.sync.dma_start(out=bq_s, in_=bq[:, :])
                nc.gpsimd.dma_start(out=relsb[0:64, :], in_=rel[:, :])
                nc.gpsimd.dma_start(out=relsb[64:128, :], in_=rel[:, :])
                nc.sync.dma_start(out=wosb, in_=wo[:, :])

                for nchunk in range(BL // 512):
                    sl = slice(nchunk * 512, (nchunk + 1) * 512)
                    for w_s, dst, b_s in ((wq_s, qT, bq_s), (wk_s, kT, None)):
                        ps = psA.tile([128, 512], F32, tag="psA")
                        for kk in range(8):
                            nc.tensor.matmul(ps, lhsT=w_s[:, kk, :], rhs=xts[kk][:, sl],
                                             start=(kk == 0), stop=(kk == 7))
                        if b_s is not None:
                            nc.vector.tensor_scalar_add(dst[:, sl], ps, b_s)
                        else:
                            nc.scalar.copy(out=dst[:, sl], in_=ps)

                for m in range(BL // 128):
                    ps = psV.tile([128, 128], F32, tag="psV")
                    msl = slice(m * 128, (m + 1) * 128)
                    for kk in range(8):
                        nc.tensor.matmul(ps, lhsT=xts[kk][:, msl], rhs=wv_s[:, kk, :],
                                         start=(kk == 0), stop=(kk == 7))
                    nc.vector.tensor_copy(
                        out=vsb[:, m, :, 0:64], in_=ps[:, :].rearrange("p (h e) -> p h e", h=2))

            # ---- cast q/k to fp8 and repack for DoubleRow ----
            with tc.tile_pool(name="tmp8", bufs=1) as tmp8p:
                q8t = tmp8p.tile([128, BL], FP8, tag="q8t")
                k8t = tmp8p.tile([128, BL], FP8, tag="k8t")
                nc.vector.tensor_copy(out=q8t, in_=qT)
                nc.scalar.copy(out=k8t, in_=kT)
                for tmp, pack in ((q8t, q8p), (k8t, k8p)):
                    for h in range(2):
                        for hf in range(2):
                            eng = nc.sync if hf == 0 else nc.gpsimd
                            p0 = h * 64 + hf * 32
                            eng.dma_start(out=pack[32 * h:32 * h + 32, hf, :],
                                          in_=tmp[p0:p0 + 32, :])

            # ---- phase B: attention + O-projection, i-tiles in pairs ----
            with (
                tc.tile_pool(name="psQE", bufs=2, space="PSUM") as psQE,
                tc.tile_pool(name="psST", bufs=2, space="PSUM") as psST,
                tc.tile_pool(name="psPV", bufs=2, space="PSUM") as psPV,
                tc.tile_pool(name="psAT", bufs=1, space="PSUM") as psAT,
                tc.tile_pool(name="psO", bufs=1, space="PSUM") as psO,
            ):
                def emit_band_skew(b, lt, h):
                    """Band matmuls + sheared SBUF->SBUF skew for one head/tile."""
                    pmin = (MAX_LEN - 128) - lt * 128
                    hsl = slice(h * 64, (h + 1) * 64)
                    lq = qT[hsl, b * L + lt * 128: b * L + lt * 128 + 128]
                    qe = qep.tile([128, BAND], F16, tag="qe")
                    for c in range(5):
                        w = 512 if c < 4 else BAND - 4 * 512
                        ps = psQE.tile([128, 512], F32, tag="psQE")
                        nc.tensor.matmul(ps[:, :w], lhsT=lq,
                                         rhs=relsb[hsl, pmin + c * 512: pmin + c * 512 + w],
                                         start=True, stop=True)
                        if c == 1:
                            nc.scalar.copy(out=qe[:, c * 512: c * 512 + w], in_=ps[:, :w])
                        else:
                            nc.vector.tensor_copy(out=qe[:, c * 512: c * 512 + w], in_=ps[:, :w])
                    rsh = rshp.tile([128, L], F16, tag="rsh")
                    nc.gpsimd.dma_start(
                        out=rsh,
                        in_=bass.AP(tensor=qe.tensor, offset=qe.offset + 127,
                                    ap=[[BAND - 1, 128], [1, L]]))
                    return rsh

                def emit_st(b, lt0, h, rsh0, rsh1):
                    """S^T for an i-pair of one head; exp evacuates P^T (bf16)."""
                    psl = slice(32 * h, 32 * h + 32)
                    lq8 = q8p[psl, :, b * L + lt0 * 128: b * L + lt0 * 128 + 256]
                    pbt = pbtp.tile([128, NT, 256], BF16, tag="pbt")
                    for g in range(8):
                        ps = psST.tile([128, 2, 256], F32, tag="psST")
                        for jj in range(2):
                            jt = g * 2 + jj
                            nc.tensor.matmul(
                                ps[:, jj, :],
                                lhsT=k8p[psl, :, b * L + jt * 128: b * L + (jt + 1) * 128],
                                rhs=lq8, perf_mode=DR,
                                start=(jj == 0), stop=False,
                                skip_group_check=True)
                            for ii, rsh in ((0, rsh0), (1, rsh1)):
                                nc.tensor.matmul(
                                    ps[:, jj, ii * 128:(ii + 1) * 128],
                                    lhsT=rsh[:, jt * 128:(jt + 1) * 128],
                                    rhs=ident, start=False,
                                    stop=(jj == 1 and ii == 1),
                                    skip_group_check=True)
                        nc.scalar.activation(out=pbt[:, g * 2:(g + 1) * 2, :],
                                             in_=ps, func=Exp)
                    return pbt

                def emit_pv(b, lt, h, ii, pbt, atps):
                    """PV + normalization + transpose into atps[h*64:(h+1)*64]."""
                    hsl = slice(h * 64, (h + 1) * 64)
                    pv = psPV.tile([128, 65], F32, tag="pv")
                    for jt in range(NT):
                        nc.tensor.matmul(pv, lhsT=pbt[:, jt, ii * 128:(ii + 1) * 128],
                                         rhs=vsb[:, b * NT + jt, h, :],
                                         start=(jt == 0), stop=(jt == NT - 1))
                    rz = small.tile([128, 1], F32, tag="rz")
                    nc.vector.reciprocal(rz, pv[:, 64:65])
                    attn = attnp.tile([128, 64], F16, tag="attn")
                    nc.vector.tensor_scalar_mul(attn, pv[:, 0:64], rz)
                    nc.tensor.matmul(atps[hsl, :], lhsT=attn, rhs=ident,
                                     start=True, stop=True, skip_group_check=True)

                def emit_out(b, lt, atps):
                    attT = attTp.tile([128, 128], F16, tag="attT")
                    nc.scalar.copy(out=attT, in_=atps)
                    osb = outp.tile([128, D], F16, tag="osb")
                    for c in range(2):
                        csl = slice(c * 512, (c + 1) * 512)
                        ops = psO.tile([128, 512], F32, tag="psO")
                        nc.tensor.matmul(ops, lhsT=attT, rhs=wosb[:, csl],
                                         start=True, stop=True)
                        nc.vector.tensor_copy(out=osb[:, csl], in_=ops)
                    nc.sync.dma_start(out=out[b * L + lt * 128: b * L + lt * 128 + 128, :],
                                      in_=osb)

                def emit_bands(b, lt0):
                    return [emit_band_skew(b, lt0 + ii, h)
                            for h in range(2) for ii in range(2)]

                order = [(b, lt0) for b in range(B) for lt0 in range(0, NT, 2)]
                pend = emit_bands(*order[0])
                for i, (b, lt0) in enumerate(order):
                    nxt = emit_bands(*order[i + 1]) if i + 1 < len(order) else None
                    r00, r01, r10, r11 = pend  # [h0i0, h0i1, h1i0, h1i1]
                    pbt0 = emit_st(b, lt0, 0, r00, r01)
                    pbt1 = emit_st(b, lt0, 1, r10, r11)
                    atps0 = psAT.tile([128, 128], F32, tag="atps")
                    atps1 = psAT.tile([128, 128], F32, tag="atps")
                    emit_pv(b, lt0, 0, 0, pbt0, atps0)
                    emit_pv(b, lt0 + 1, 0, 1, pbt0, atps1)
                    emit_pv(b, lt0, 1, 0, pbt1, atps0)
                    emit_out(b, lt0, atps0)
                    emit_pv(b, lt0 + 1, 1, 1, pbt1, atps1)
                    emit_out(b, lt0 + 1, atps1)
                    pend = nxt
    nc.compile()
    return nc


def make_in_maps(inputs):
    x = np.asarray(inputs["x"], np.float32)
    Wq = np.asarray(inputs["Wq"], np.float32)
    bq = np.asarray(inputs["bq"], np.float32)
    Wk = np.asarray(inputs["Wk"], np.float32)
    Wv = np.asarray(inputs["Wv"], np.float32)
    Wo = np.asarray(inputs["Wo"], np.float32)
    rel = np.asarray(inputs["rel_emb"], np.float32)

    s8 = 1.0 / np.sqrt(HD)
    xT = np.ascontiguousarray(x.reshape(BL, D).T).astype(NPF16)
    rel8T = np.zeros((HD, RELW), NPF16)
    rel8T[:, :2 * MAX_LEN - 1] = (rel.T / s8).astype(NPF16)

    in_maps = []
    for c in range(NCORES):
        E = slice(EPC * c, EPC * (c + 1))
        in_maps.append({
            "xT": xT,
            "wqT8": np.ascontiguousarray((Wq[E, :] * s8).T).astype(NPF16),
            "wkT": np.ascontiguousarray(Wk[E, :].T).astype(NPF16),
            "wvT": np.ascontiguousarray(Wv[E, :].T).astype(NPF16),
            "woT": np.ascontiguousarray(Wo[:, E].T).astype(NPF16),
            "rel8T": rel8T,
            "bq8": (bq[E] * s8).astype(np.float32).reshape(EPC, 1),
        })
    return in_maps


def host_const(inputs):
    bv = np.asarray(inputs["bv"], np.float64)
    Wo = np.asarray(inputs["Wo"], np.float64)
    bo = np.asarray(inputs["bo"], np.float64)
    return bv @ Wo.T + bo


_NC_CACHE = None


def get_nc():
    global _NC_CACHE
    if _NC_CACHE is None:
        _NC_CACHE = build_bass()
    return _NC_CACHE


def kernel(**inputs):
    nc = get_nc()
    in_maps = make_in_maps(inputs)
    res = run_bass_kernel_spmd(nc, in_maps, core_ids=list(range(NCORES)))
    acc = np.zeros((BL, D), np.float64)
    for c in range(NCORES):
        acc += res.results[c]["out"].astype(np.float64)
    acc += host_const(inputs)
    return acc.reshape(B, L, D).astype(np.float32)
